# revision 19
# baseline (speedup 1.0000x reference)
"""Trainium2 Bass kernel for additive (Bahdanau) attention.

reference computation (B=4, Q=K=512, D=256, H=128, V=256):
    fq = queries @ wq_w.T + wq_b                    # [B,Q,H]
    fk = keys @ wk_w.T + wk_b                       # [B,K,H]
    scores[b,q,k] = sum_h wv[h]*tanh(fq[b,q,h]+fk[b,k,h]) + wv_b
    attn = softmax(mask(scores, valid_lens), axis=k)
    out  = attn @ values                            # [B,Q,V]

Sharding: 8 cores = 4 batches x 2 query-halves; zero cross-core traffic.

Key specialization: positions >= valid_len get attention weight exactly
0 (additive -1e6 mask -> f32 exp underflows to 0.0), so the graph is
compiled for KC = ceil(max(valid_lens)/8)*8 key positions (compile
cache per KC). Exact and input-adaptive; cuts the dominant per-element
tanh work proportionally.

Per-core device algorithm (H=128 on partitions; ScalarE tanh throughput
of 128 lanes * 1.2 GHz is the fundamental bound, so every other engine
is arranged to hide under it):
  - fqT[h,q], fkT[h,k] via PE matmuls in bf16 (inputs pre-transposed
    host-side, packed into two bf16 DMAs; fq first so the DVE bias-fold
    overlaps the fk matmuls).
  - tanh features: per q a [128h, KC] tile of tanh(fkT + fqT[:,q]).
    The first 8 q of block 0 run as ACT activations with per-partition
    bias straight out of the fk PSUM (no DVE dependency -> ACT starts
    ~4us earlier); the rest are DVE tensor_scalar adds in bf16 (4x
    mode, ~220ns per q) batched up to 32 q per ACT tanh call, with
    ramped supertile sizes at the start/end of the q range.
  - scores [128q, KC] accumulate in PSUM, one bank per 64-q half so
    each half's softmax overlaps the other half's matmuls: first a
    rank-1 matmul writes the additive mask row (start=True clears the
    bank), then per q one matmul with a one-hot-weighted wv column
    (lhsT = wv (x) e_j, M=32 col-group tiles, built on device from wv)
    accumulates score row q.
  - softmax without max-subtraction (|scores| <= sum|wv| ~ 9): ACT exp
    -> E f32; DVE row-sum + reciprocal; masked lanes are exactly 0, so
    the result matches the reference's masked softmax bit-for-bit in
    the masked positions.
  - attn^T via PE transposes, attn @ values on PE in bf16, per-row
    1/denom scale fused into the PSUM->SBUF copy; the final q-half's
    epilogue runs on ScalarE (otherwise idle) to shorten the tail.
"""

import sys

sys.path.insert(0, "/opt/trn_rl_repo")

import contextlib
from contextlib import ExitStack

import ml_dtypes
import numpy as np

from concourse import bacc, mybir, tile
from concourse.bass_utils import run_bass_kernel_spmd
from concourse.masks import make_identity
from concourse.tile_rust import add_dep_helper

B, Q, K, D, H, V = 4, 512, 512, 256, 128, 256
QS = Q // 2          # query rows per core
NCORES = 8
MASK_VALUE = -1000000.0

f32 = mybir.dt.float32
bf16 = mybir.dt.bfloat16

# (kind, q-count) per supertile; block 0 leads with ACT-biased q's and
# ramps up, last block ramps down to shorten the serial tail.
STS0 = [("bias", 8), ("bat", 16), ("bat", 24), ("bat", 32), ("bat", 32),
        ("bat", 16)]
STS1 = [("bat", 32), ("bat", 32), ("bat", 32), ("bat", 16), ("bat", 8),
        ("bat", 4), ("bat", 4)]


def _build_graph(nc, tc, ctx, tensors, KC):
    pA_d, pB_d, p2_d, wb_d, m_d, z_d, out_d = tensors
    NKC = (KC + 127) // 128          # 128-row key chunks (last may be partial)
    WLAST = KC - (NKC - 1) * 128     # rows in last chunk
    Tanh = mybir.ActivationFunctionType.Tanh
    Exp = mybir.ActivationFunctionType.Exp
    AX = mybir.AxisListType.X
    ADD = mybir.AluOpType.add

    cpool = ctx.enter_context(tc.tile_pool(name="const", bufs=1))
    inp = ctx.enter_context(tc.tile_pool(name="inp", bufs=1))
    stbufs = 3 if KC <= 416 else 2
    prep = ctx.enter_context(tc.tile_pool(name="prep", bufs=stbufs))
    ttp = ctx.enter_context(tc.tile_pool(name="ttp", bufs=stbufs))
    smp = ctx.enter_context(tc.tile_pool(name="smp", bufs=2))
    outp = ctx.enter_context(tc.tile_pool(name="outp", bufs=2))
    ps_big = ctx.enter_context(tc.tile_pool(name="ps_big", bufs=4, space="PSUM"))
    ps_tr = ctx.enter_context(tc.tile_pool(name="ps_tr", bufs=2, space="PSUM"))
    ps_av = ctx.enter_context(tc.tile_pool(name="ps_av", bufs=2, space="PSUM"))

    # ---------------- constants built before DVE gets busy ----------------
    ident = cpool.tile([128, 128], f32, tag="ident")
    make_identity(nc, ident[:])
    ones_bf = cpool.tile([1, H], bf16, tag="ones")
    nc.gpsimd.memset(ones_bf[:], 1.0)

    # ---------------- loads ----------------
    pkB = inp.tile([128, 768], bf16, tag="pkB")
    nc.sync.dma_start(pkB[:], pB_d[:])
    qT_sb = [pkB[:, i * 256:(i + 1) * 256] for i in range(2)]
    wqT_sb = [pkB[:, 512 + i * 128:512 + (i + 1) * 128] for i in range(2)]
    FA = 2 * KC + 256
    pkA = inp.tile([128, FA], bf16, tag="pkA")
    nc.sync.dma_start(pkA[:], pA_d[:])
    kT_sb = [pkA[:, i * KC:(i + 1) * KC] for i in range(2)]
    wkT_sb = [pkA[:, 2 * KC + i * 128:2 * KC + (i + 1) * 128] for i in range(2)]
    wb_sb = inp.tile([128, 2], f32, tag="wb")
    nc.sync.dma_start(wb_sb[:], wb_d[:])
    wqkb = wb_sb[:, 0:1]

    mask_bf = cpool.tile([1, KC], bf16, tag="maskbf")
    nc.sync.dma_start(mask_bf[:], m_d[:])

    # one-hot weighted wv columns z32[h, j*32+m] = wv[h] iff m == j,
    # prebuilt host-side (a strided on-device build costs ~1.2us on DVE
    # right in the startup critical path)
    z32 = cpool.tile([128, 1024], bf16, tag="z32")
    nc.sync.dma_start(z32[:], z_d[:])

    vals = inp.tile([128, NKC * V], f32, tag="vals")
    nc.sync.dma_start(vals[:], p2_d[:])
    vals_bf = cpool.tile([128, NKC * V], bf16, tag="vals_bf")

    # ---------------- projections (fq first: DVE bias-add overlaps fk) ----
    fq_ps = ps_big.tile([128, QS], f32, tag="big", name="fq_ps")
    nc.tensor.matmul(fq_ps[:], wqT_sb[0], qT_sb[0], start=True, stop=False)
    nc.tensor.matmul(fq_ps[:], wqT_sb[1], qT_sb[1], start=False, stop=True)
    fq_sb = cpool.tile([128, QS], f32, tag="fq_sb")
    # fold wq_b + wk_b into fq so the tanh input needs no extra bias
    fq_add = nc.vector.tensor_scalar_add(fq_sb[:], fq_ps[:], wqkb)

    fk_ps = ps_big.tile([128, K], f32, tag="big", name="fk_ps")
    nc.tensor.matmul(fk_ps[:, :KC], wkT_sb[0], kT_sb[0], start=True, stop=False)
    nc.tensor.matmul(fk_ps[:, :KC], wkT_sb[1], kT_sb[1], start=False, stop=True)
    fk_sb = cpool.tile([128, KC], bf16, tag="fk_sb")
    fk_cast = nc.vector.tensor_copy(fk_sb[:], fk_ps[:, :KC])
    add_dep_helper(fk_cast.ins, fq_add.ins, sync=False,
                   reason="fq bias-add first on DVE")

    # ---------------- main loop ----------------
    for blk in range(2):
        sts = STS0 if blk == 0 else STS1
        # one PSUM bank per 64-q half so softmax of half A overlaps the
        # score matmuls of half B
        sc_a = ps_big.tile([128, K], f32, tag="big", name=f"sc{blk}a")
        sc_b = ps_big.tile([128, K], f32, tag="big", name=f"sc{blk}b")
        # rank-1 matmul writes mask row to every q-partition, clears bank
        nc.tensor.matmul(sc_a[:, :KC], ones_bf[:], mask_bf[:], start=True,
                         stop=False, skip_group_check=True)
        nc.tensor.matmul(sc_b[:, :KC], ones_bf[:], mask_bf[:], start=True,
                         stop=False, skip_group_check=True)
        r = 0
        biased_acts = []
        for st, (kind, stq) in enumerate(sts):
            tt = ttp.tile([128, 32 * KC], bf16, tag="tt", name=f"tt{blk}_{st}")
            if kind == "bias":
                # ACT reads fk straight from PSUM, per-partition bias fq[:,q]
                with tc.high_priority():
                    for i in range(stq):
                        q = blk * 128 + r + i
                        biased_acts.append(nc.scalar.activation(
                            tt[:, i * KC:(i + 1) * KC], fk_ps[:, :KC], Tanh,
                            bias=fq_sb[:, q:q + 1]))
            else:
                pre = prep.tile([128, 32 * KC], bf16, tag="pre",
                                name=f"pre{blk}_{st}")
                # small trailing supertiles: keep their adds ahead of the
                # concurrent softmax DVE work so ACT never starves
                prio = (tc.high_priority() if blk == 1 and stq <= 8
                        else contextlib.nullcontext())
                with prio:
                    for i in range(stq):
                        q = blk * 128 + r + i
                        nc.vector.tensor_scalar_add(
                            pre[:, i * KC:(i + 1) * KC], fk_sb[:],
                            fq_sb[:, q:q + 1])
                bat = nc.scalar.activation(tt[:, :stq * KC], pre[:, :stq * KC],
                                           Tanh)
                # keep the ACT stream in biased -> batched order at startup
                for bi in biased_acts:
                    add_dep_helper(bat.ins, bi.ins, sync=False,
                                   reason="biased tanhs precede batched")
                biased_acts = []
            for i in range(stq):
                g, j = divmod(r + i, 32)
                sc = sc_a if g < 2 else sc_b
                nc.tensor.matmul(
                    sc[g * 32:(g + 1) * 32, :KC],
                    z32[:, j * 32:(j + 1) * 32],
                    tt[:, i * KC:(i + 1) * KC],
                    start=False, stop=(r + i in (63, 127)),
                    skip_group_check=True, tile_position=(0, g * 32))
            r += stq

        if blk == 0:
            # off the critical path: cast values to bf16 for the AV matmuls
            nc.vector.tensor_copy(vals_bf[:], vals[:])
        for hi, sc in enumerate((sc_a, sc_b)):
            q0 = hi * 64
            E = smp.tile([64, KC], f32, tag=f"E{hi}", name=f"E{blk}_{hi}")
            nc.scalar.activation(E[:], sc[q0:q0 + 64, :KC], Exp)
            denom = smp.tile([64, 1], f32, tag=f"dn{hi}", name=f"dn{blk}_{hi}")
            nc.vector.tensor_reduce(denom[:], E[:], axis=AX, op=ADD)
            recip = smp.tile([64, 1], f32, tag=f"rc{hi}", name=f"rc{blk}_{hi}")
            nc.vector.reciprocal(recip[:], denom[:])

            ET = smp.tile([128, NKC * 64], bf16, tag=f"ET{hi}",
                          name=f"ET{blk}_{hi}")
            for ci in range(NKC):
                w = 128 if ci < NKC - 1 else WLAST
                tp = ps_tr.tile([128, 128], f32, tag="tr",
                                name=f"tr{blk}_{hi}_{ci}")
                nc.tensor.transpose(tp[:w, :64], E[:, ci * 128:ci * 128 + w],
                                    ident[0:64, 0:64])
                if blk == 1 and hi == 1:
                    nc.scalar.copy(ET[:w, ci * 64:ci * 64 + 64], tp[:w, :64])
                else:
                    nc.vector.tensor_copy(ET[:w, ci * 64:ci * 64 + 64],
                                          tp[:w, :64])

            av = ps_av.tile([64, V], f32, tag="av", name=f"av{blk}_{hi}")
            for ci in range(NKC):
                w = 128 if ci < NKC - 1 else WLAST
                nc.tensor.matmul(av[:], ET[:w, ci * 64:ci * 64 + 64],
                                 vals_bf[:w, ci * V:(ci + 1) * V],
                                 start=(ci == 0), stop=(ci == NKC - 1))
            osb = outp.tile([64, V], f32, tag=f"osb{hi}",
                            name=f"osb{blk}_{hi}")
            if blk == 1 and hi == 1:
                nc.scalar.activation(osb[:], av[:],
                                     mybir.ActivationFunctionType.Copy,
                                     scale=recip[:])
            else:
                nc.vector.tensor_scalar_mul(osb[:], av[:], recip[:])
            nc.sync.dma_start(out_d[blk * 128 + q0:blk * 128 + q0 + 64, :],
                              osb[:])


def _build_kernel(KC):
    NKC = (KC + 127) // 128
    nc = bacc.Bacc("TRN2", target_bir_lowering=False, debug=False,
                   num_devices=NCORES, enable_partition_id=False)
    pA_d = nc.dram_tensor("packA", [128, 2 * KC + 256], bf16,
                          kind="ExternalInput")
    pB_d = nc.dram_tensor("packB", [128, 768], bf16, kind="ExternalInput")
    p2_d = nc.dram_tensor("pack2", [128, NKC * V], f32, kind="ExternalInput")
    wb_d = nc.dram_tensor("wb", [128, 2], f32, kind="ExternalInput")
    m_d = nc.dram_tensor("maskrow", [1, KC], bf16, kind="ExternalInput")
    z_d = nc.dram_tensor("z32", [128, 1024], bf16, kind="ExternalInput")
    out_d = nc.dram_tensor("out", [QS, V], f32, kind="ExternalOutput")

    with tile.TileContext(nc) as tc, ExitStack() as ctx:
        _build_graph(nc, tc, ctx, (pA_d, pB_d, p2_d, wb_d, m_d, z_d, out_d),
                     KC)
    nc.compile()
    return nc


_NC_CACHE = {}


def _get_nc(KC):
    if KC not in _NC_CACHE:
        _NC_CACHE[KC] = _build_kernel(KC)
    return _NC_CACHE[KC]


def _choose_kc(valid_lens):
    mx = int(np.max(valid_lens))
    mx = max(32, min(K, mx))
    return (mx + 7) // 8 * 8


def prepare_in_maps(queries, keys, values, valid_lens, wq_w, wq_b, wk_w,
                    wk_b, wv_w, wv_b):
    queries = np.asarray(queries, np.float32)
    keys = np.asarray(keys, np.float32)
    values = np.asarray(values, np.float32)
    wq_w = np.asarray(wq_w, np.float32)
    wq_b = np.asarray(wq_b, np.float32)
    wk_w = np.asarray(wk_w, np.float32)
    wk_b = np.asarray(wk_b, np.float32)
    wv_w = np.asarray(wv_w, np.float32)
    wv_b = np.asarray(wv_b, np.float32)
    valid_lens = np.asarray(valid_lens)

    KC = _choose_kc(valid_lens)
    NKC = (KC + 127) // 128

    wqT = wq_w.T                     # [D, H]
    wkT = wk_w.T
    wqkb = (wq_b + wk_b).reshape(H, 1)
    wv = wv_w.reshape(H, 1)
    # z32[h, j*32+m] = wv[h] iff m == j
    z32 = np.zeros((H, 1024), np.float32)
    for j in range(32):
        z32[:, j * 33] = wv[:, 0]
    z32 = z32.astype(ml_dtypes.bfloat16)

    in_maps = []
    for c in range(NCORES):
        b, half = divmod(c, 2)
        vl = int(valid_lens[b])
        mask = np.full((1, KC), MASK_VALUE, np.float32)
        mask[0, :vl] = 0.0
        mask += np.float32(wv_b.reshape(-1)[0])
        mask = mask.astype(ml_dtypes.bfloat16)

        kT = keys[b, :KC, :].T                            # [D, KC]
        qT = queries[b, half * QS:(half + 1) * QS, :].T   # [D, QS]
        packA = np.concatenate([
            kT[0:128], kT[128:256],
            wkT[0:128], wkT[128:256],
        ], axis=1).astype(ml_dtypes.bfloat16)
        packB = np.concatenate([
            qT[0:128], qT[128:256],
            wqT[0:128], wqT[128:256],
        ], axis=1).astype(ml_dtypes.bfloat16)
        wb = np.concatenate([wqkb, wv], axis=1).astype(np.float32)

        vpad = np.zeros((NKC * 128, V), np.float32)
        vpad[:KC] = values[b, :KC, :]
        pack2 = np.concatenate(
            [vpad[ci * 128:(ci + 1) * 128] for ci in range(NKC)], axis=1)

        in_maps.append({
            "packA": np.ascontiguousarray(packA),
            "packB": np.ascontiguousarray(packB),
            "pack2": np.ascontiguousarray(pack2),
            "wb": np.ascontiguousarray(wb),
            "maskrow": mask,
            "z32": z32,
        })
    return KC, in_maps


def assemble_out(results):
    out = np.empty((B, Q, V), np.float32)
    for c in range(NCORES):
        b, half = divmod(c, 2)
        out[b, half * QS:(half + 1) * QS, :] = results[c]["out"]
    return out


def kernel(**inputs):
    KC, in_maps = prepare_in_maps(**inputs)
    nc = _get_nc(KC)
    try:
        res = run_bass_kernel_spmd(nc, in_maps, list(range(NCORES))).results
    except Exception:
        # transient NRT/device hiccups happen; one retry
        import time
        time.sleep(2.0)
        res = run_bass_kernel_spmd(nc, in_maps, list(range(NCORES))).results
    return assemble_out(res)


if __name__ == "__main__":
    rng = np.random.default_rng(0)
    inp = {
        "queries": rng.standard_normal((B, Q, D), np.float32),
        "keys": rng.standard_normal((B, K, D), np.float32),
        "values": rng.standard_normal((B, K, V), np.float32),
        "valid_lens": rng.integers(1, K + 1, (B,)).astype(np.int32),
        "wq_w": (rng.standard_normal((H, D), np.float32) / np.sqrt(D)).astype(np.float32),
        "wq_b": np.zeros((H,), np.float32),
        "wk_w": (rng.standard_normal((H, D), np.float32) / np.sqrt(D)).astype(np.float32),
        "wk_b": np.zeros((H,), np.float32),
        "wv_w": (rng.standard_normal((1, H), np.float32) / np.sqrt(H)).astype(np.float32),
        "wv_b": np.zeros((1,), np.float32),
    }
    out = kernel(**inp)
    print("kernel output", out.shape, out.dtype, float(np.abs(out).mean()))


# revision 20
# speedup vs baseline: 1.1877x; 1.1877x over previous
"""Trainium2 Bass kernel for additive (Bahdanau) attention.

reference computation (B=4, Q=K=512, D=256, H=128, V=256):
    fq = queries @ wq_w.T + wq_b                    # [B,Q,H]
    fk = keys @ wk_w.T + wk_b                       # [B,K,H]
    scores[b,q,k] = sum_h wv[h]*tanh(fq[b,q,h]+fk[b,k,h]) + wv_b
    attn = softmax(mask(scores, valid_lens), axis=k)
    out  = attn @ values                            # [B,Q,V]

Sharding: 8 cores = 4 batches x 2 query-halves; zero cross-core traffic.

Key specialization: positions >= valid_len get attention weight exactly
0 (additive -1e6 mask -> f32 exp underflows to 0.0), so the graph is
compiled for KC = ceil(max(valid_lens)/8)*8 key positions (compile
cache per KC). Exact and input-adaptive; cuts the dominant per-element
tanh work proportionally.

Per-core device algorithm (H=128 on partitions; ScalarE tanh throughput
of 128 lanes * 1.2 GHz is the fundamental bound, so every other engine
is arranged to hide under it):
  - fqT[h,q], fkT[h,k] via PE matmuls in bf16 (inputs pre-transposed
    host-side, packed into two bf16 DMAs; fq first so the DVE bias-fold
    overlaps the fk matmuls).
  - tanh features: per q a [128h, KC] tile of tanh(fkT + fqT[:,q]).
    The first 8 q of block 0 run as ACT activations with per-partition
    bias straight out of the fk PSUM (no DVE dependency -> ACT starts
    ~4us earlier); the rest are DVE tensor_scalar adds in bf16 (4x
    mode, ~220ns per q) batched up to 32 q per ACT tanh call, with
    ramped supertile sizes at the start/end of the q range.
  - scores [128q, KC] accumulate in PSUM, one bank per 64-q half so
    each half's softmax overlaps the other half's matmuls: first a
    rank-1 matmul writes the additive mask row (start=True clears the
    bank), then per q one matmul with a one-hot-weighted wv column
    (lhsT = wv (x) e_j, M=32 col-group tiles, built on device from wv)
    accumulates score row q.
  - softmax without max-subtraction (|scores| <= sum|wv| ~ 9): ACT exp
    -> E f32; DVE row-sum + reciprocal; masked lanes are exactly 0, so
    the result matches the reference's masked softmax bit-for-bit in
    the masked positions.
  - attn^T via PE transposes, attn @ values on PE in bf16, per-row
    1/denom scale fused into the PSUM->SBUF copy; the final q-half's
    epilogue runs on ScalarE (otherwise idle) to shorten the tail.
"""

import sys

sys.path.insert(0, "/opt/trn_rl_repo")

import contextlib
from contextlib import ExitStack

import ml_dtypes
import numpy as np

from concourse import bacc, mybir, tile
from concourse.bass_utils import run_bass_kernel_spmd
from concourse.masks import make_identity
from concourse.tile_rust import add_dep_helper

B, Q, K, D, H, V = 4, 512, 512, 256, 128, 256
QS = Q // 2          # query rows per core
NCORES = 8
MASK_VALUE = -1000000.0

f32 = mybir.dt.float32
bf16 = mybir.dt.bfloat16

# (kind, q-count) per supertile; block 0 leads with ACT-biased q's and
# ramps up, last block ramps down to shorten the serial tail.
STS0 = [("bias", 8), ("bat", 16), ("bat", 24), ("bat", 32), ("bat", 32),
        ("bat", 16)]
STS1 = [("bat", 32), ("bat", 32), ("bat", 32), ("bat", 16), ("bat", 8),
        ("bat", 4), ("bat", 4)]


def _build_graph(nc, tc, ctx, tensors, KC):
    pA_d, pB_d, p2_d, wb_d, m_d, z_d, out_d = tensors
    NKC = (KC + 127) // 128          # 128-row key chunks (last may be partial)
    WLAST = KC - (NKC - 1) * 128     # rows in last chunk
    Tanh = mybir.ActivationFunctionType.Tanh
    Exp = mybir.ActivationFunctionType.Exp
    AX = mybir.AxisListType.X
    ADD = mybir.AluOpType.add

    cpool = ctx.enter_context(tc.tile_pool(name="const", bufs=1))
    inp = ctx.enter_context(tc.tile_pool(name="inp", bufs=1))
    stbufs = 3 if KC <= 416 else 2
    prep = ctx.enter_context(tc.tile_pool(name="prep", bufs=stbufs))
    ttp = ctx.enter_context(tc.tile_pool(name="ttp", bufs=stbufs))
    smp = ctx.enter_context(tc.tile_pool(name="smp", bufs=2))
    outp = ctx.enter_context(tc.tile_pool(name="outp", bufs=2))
    ps_big = ctx.enter_context(tc.tile_pool(name="ps_big", bufs=4, space="PSUM"))
    ps_tr = ctx.enter_context(tc.tile_pool(name="ps_tr", bufs=2, space="PSUM"))
    ps_av = ctx.enter_context(tc.tile_pool(name="ps_av", bufs=2, space="PSUM"))

    # ---------------- constants built before DVE gets busy ----------------
    ident = cpool.tile([128, 128], f32, tag="ident")
    make_identity(nc, ident[:])
    ones_bf = cpool.tile([1, H], bf16, tag="ones")
    nc.gpsimd.memset(ones_bf[:], 1.0)

    # ---------------- loads ----------------
    pkB = inp.tile([128, 768], bf16, tag="pkB")
    nc.sync.dma_start(pkB[:], pB_d[:])
    qT_sb = [pkB[:, i * 256:(i + 1) * 256] for i in range(2)]
    wqT_sb = [pkB[:, 512 + i * 128:512 + (i + 1) * 128] for i in range(2)]
    FA = 2 * KC + 256
    pkA = inp.tile([128, FA], bf16, tag="pkA")
    nc.sync.dma_start(pkA[:], pA_d[:])
    kT_sb = [pkA[:, i * KC:(i + 1) * KC] for i in range(2)]
    wkT_sb = [pkA[:, 2 * KC + i * 128:2 * KC + (i + 1) * 128] for i in range(2)]
    wb_sb = inp.tile([128, 2], f32, tag="wb")
    nc.sync.dma_start(wb_sb[:], wb_d[:])
    wqkb = wb_sb[:, 0:1]

    mask_bf = cpool.tile([1, KC], bf16, tag="maskbf")
    nc.sync.dma_start(mask_bf[:], m_d[:])

    # one-hot weighted wv columns z32[h, j*32+m] = wv[h] iff m == j,
    # prebuilt host-side (a strided on-device build costs ~1.2us on DVE
    # right in the startup critical path)
    z32 = cpool.tile([128, 1024], bf16, tag="z32")
    nc.sync.dma_start(z32[:], z_d[:])

    vals = inp.tile([128, NKC * V], f32, tag="vals")
    nc.sync.dma_start(vals[:], p2_d[:])
    vals_bf = cpool.tile([128, NKC * V], bf16, tag="vals_bf")

    # ---------------- projections (fq first: DVE bias-add overlaps fk) ----
    fq_ps = ps_big.tile([128, QS], f32, tag="big", name="fq_ps")
    nc.tensor.matmul(fq_ps[:], wqT_sb[0], qT_sb[0], start=True, stop=False)
    nc.tensor.matmul(fq_ps[:], wqT_sb[1], qT_sb[1], start=False, stop=True)
    fq_sb = cpool.tile([128, QS], f32, tag="fq_sb")
    # fold wq_b + wk_b into fq so the tanh input needs no extra bias
    fq_add = nc.vector.tensor_scalar_add(fq_sb[:], fq_ps[:], wqkb)

    fk_ps = ps_big.tile([128, K], f32, tag="big", name="fk_ps")
    nc.tensor.matmul(fk_ps[:, :KC], wkT_sb[0], kT_sb[0], start=True, stop=False)
    nc.tensor.matmul(fk_ps[:, :KC], wkT_sb[1], kT_sb[1], start=False, stop=True)
    fk_sb = cpool.tile([128, KC], bf16, tag="fk_sb")
    fk_cast = nc.vector.tensor_copy(fk_sb[:], fk_ps[:, :KC])
    add_dep_helper(fk_cast.ins, fq_add.ins, sync=False,
                   reason="fq bias-add first on DVE")

    # ---------------- main loop ----------------
    for blk in range(2):
        sts = STS0 if blk == 0 else STS1
        # one PSUM bank per 64-q half so softmax of half A overlaps the
        # score matmuls of half B
        sc_a = ps_big.tile([128, K], f32, tag="big", name=f"sc{blk}a")
        sc_b = ps_big.tile([128, K], f32, tag="big", name=f"sc{blk}b")
        # rank-1 matmul writes mask row to every q-partition, clears bank
        nc.tensor.matmul(sc_a[:, :KC], ones_bf[:], mask_bf[:], start=True,
                         stop=False, skip_group_check=True)
        nc.tensor.matmul(sc_b[:, :KC], ones_bf[:], mask_bf[:], start=True,
                         stop=False, skip_group_check=True)
        r = 0
        biased_acts = []
        for st, (kind, stq) in enumerate(sts):
            tt = ttp.tile([128, 32 * KC], bf16, tag="tt", name=f"tt{blk}_{st}")
            if kind == "bias":
                # ACT reads fk straight from PSUM, per-partition bias fq[:,q]
                with tc.high_priority():
                    for i in range(stq):
                        q = blk * 128 + r + i
                        biased_acts.append(nc.scalar.activation(
                            tt[:, i * KC:(i + 1) * KC], fk_ps[:, :KC], Tanh,
                            bias=fq_sb[:, q:q + 1]))
            else:
                pre = prep.tile([128, 32 * KC], bf16, tag="pre",
                                name=f"pre{blk}_{st}")
                # small trailing supertiles: keep their adds ahead of the
                # concurrent softmax DVE work so ACT never starves
                prio = (tc.high_priority() if blk == 1 and stq <= 8
                        else contextlib.nullcontext())
                with prio:
                    for i in range(stq):
                        q = blk * 128 + r + i
                        nc.vector.tensor_scalar_add(
                            pre[:, i * KC:(i + 1) * KC], fk_sb[:],
                            fq_sb[:, q:q + 1])
                bat = nc.scalar.activation(tt[:, :stq * KC], pre[:, :stq * KC],
                                           Tanh)
                # keep the ACT stream in biased -> batched order at startup
                for bi in biased_acts:
                    add_dep_helper(bat.ins, bi.ins, sync=False,
                                   reason="biased tanhs precede batched")
                biased_acts = []
            for i in range(stq):
                g, j = divmod(r + i, 32)
                sc = sc_a if g < 2 else sc_b
                nc.tensor.matmul(
                    sc[g * 32:(g + 1) * 32, :KC],
                    z32[:, j * 32:(j + 1) * 32],
                    tt[:, i * KC:(i + 1) * KC],
                    start=False, stop=(r + i in (63, 127)),
                    skip_group_check=True, tile_position=(0, g * 32))
            r += stq

        if blk == 0:
            # off the critical path: cast values to bf16 for the AV matmuls
            nc.vector.tensor_copy(vals_bf[:], vals[:])
        for hi, sc in enumerate((sc_a, sc_b)):
            q0 = hi * 64
            E = smp.tile([64, KC], f32, tag=f"E{hi}", name=f"E{blk}_{hi}")
            nc.scalar.activation(E[:], sc[q0:q0 + 64, :KC], Exp)
            denom = smp.tile([64, 1], f32, tag=f"dn{hi}", name=f"dn{blk}_{hi}")
            nc.vector.tensor_reduce(denom[:], E[:], axis=AX, op=ADD)
            recip = smp.tile([64, 1], f32, tag=f"rc{hi}", name=f"rc{blk}_{hi}")
            nc.vector.reciprocal(recip[:], denom[:])

            ET = smp.tile([128, NKC * 64], bf16, tag=f"ET{hi}",
                          name=f"ET{blk}_{hi}")
            for ci in range(NKC):
                w = 128 if ci < NKC - 1 else WLAST
                tp = ps_tr.tile([128, 128], f32, tag="tr",
                                name=f"tr{blk}_{hi}_{ci}")
                nc.tensor.transpose(tp[:w, :64], E[:, ci * 128:ci * 128 + w],
                                    ident[0:64, 0:64])
                nc.vector.tensor_copy(ET[:w, ci * 64:ci * 64 + 64],
                                      tp[:w, :64])

            av = ps_av.tile([64, V], f32, tag="av", name=f"av{blk}_{hi}")
            for ci in range(NKC):
                w = 128 if ci < NKC - 1 else WLAST
                nc.tensor.matmul(av[:], ET[:w, ci * 64:ci * 64 + 64],
                                 vals_bf[:w, ci * V:(ci + 1) * V],
                                 start=(ci == 0), stop=(ci == NKC - 1))
            osb = outp.tile([64, V], f32, tag=f"osb{hi}",
                            name=f"osb{blk}_{hi}")
            nc.vector.tensor_scalar_mul(osb[:], av[:], recip[:])
            nc.sync.dma_start(out_d[blk * 128 + q0:blk * 128 + q0 + 64, :],
                              osb[:])


def _build_kernel(KC):
    NKC = (KC + 127) // 128
    nc = bacc.Bacc("TRN2", target_bir_lowering=False, debug=False,
                   num_devices=NCORES, enable_partition_id=False)
    pA_d = nc.dram_tensor("packA", [128, 2 * KC + 256], bf16,
                          kind="ExternalInput")
    pB_d = nc.dram_tensor("packB", [128, 768], bf16, kind="ExternalInput")
    p2_d = nc.dram_tensor("pack2", [128, NKC * V], f32, kind="ExternalInput")
    wb_d = nc.dram_tensor("wb", [128, 2], f32, kind="ExternalInput")
    m_d = nc.dram_tensor("maskrow", [1, KC], bf16, kind="ExternalInput")
    z_d = nc.dram_tensor("z32", [128, 1024], bf16, kind="ExternalInput")
    out_d = nc.dram_tensor("out", [QS, V], f32, kind="ExternalOutput")

    with tile.TileContext(nc) as tc, ExitStack() as ctx:
        _build_graph(nc, tc, ctx, (pA_d, pB_d, p2_d, wb_d, m_d, z_d, out_d),
                     KC)
    nc.compile()
    return nc


_NC_CACHE = {}


def _get_nc(KC):
    if KC not in _NC_CACHE:
        _NC_CACHE[KC] = _build_kernel(KC)
    return _NC_CACHE[KC]


def _choose_kc(valid_lens):
    mx = int(np.max(valid_lens))
    mx = max(32, min(K, mx))
    return (mx + 7) // 8 * 8


def prepare_in_maps(queries, keys, values, valid_lens, wq_w, wq_b, wk_w,
                    wk_b, wv_w, wv_b):
    queries = np.asarray(queries, np.float32)
    keys = np.asarray(keys, np.float32)
    values = np.asarray(values, np.float32)
    wq_w = np.asarray(wq_w, np.float32)
    wq_b = np.asarray(wq_b, np.float32)
    wk_w = np.asarray(wk_w, np.float32)
    wk_b = np.asarray(wk_b, np.float32)
    wv_w = np.asarray(wv_w, np.float32)
    wv_b = np.asarray(wv_b, np.float32)
    valid_lens = np.asarray(valid_lens)

    KC = _choose_kc(valid_lens)
    NKC = (KC + 127) // 128

    wqT = wq_w.T                     # [D, H]
    wkT = wk_w.T
    wqkb = (wq_b + wk_b).reshape(H, 1)
    wv = wv_w.reshape(H, 1)
    # z32[h, j*32+m] = wv[h] iff m == j
    z32 = np.zeros((H, 1024), np.float32)
    for j in range(32):
        z32[:, j * 33] = wv[:, 0]
    z32 = z32.astype(ml_dtypes.bfloat16)

    in_maps = []
    for c in range(NCORES):
        b, half = divmod(c, 2)
        vl = int(valid_lens[b])
        mask = np.full((1, KC), MASK_VALUE, np.float32)
        mask[0, :vl] = 0.0
        mask += np.float32(wv_b.reshape(-1)[0])
        mask = mask.astype(ml_dtypes.bfloat16)

        kT = keys[b, :KC, :].T                            # [D, KC]
        qT = queries[b, half * QS:(half + 1) * QS, :].T   # [D, QS]
        packA = np.concatenate([
            kT[0:128], kT[128:256],
            wkT[0:128], wkT[128:256],
        ], axis=1).astype(ml_dtypes.bfloat16)
        packB = np.concatenate([
            qT[0:128], qT[128:256],
            wqT[0:128], wqT[128:256],
        ], axis=1).astype(ml_dtypes.bfloat16)
        wb = np.concatenate([wqkb, wv], axis=1).astype(np.float32)

        vpad = np.zeros((NKC * 128, V), np.float32)
        vpad[:KC] = values[b, :KC, :]
        pack2 = np.concatenate(
            [vpad[ci * 128:(ci + 1) * 128] for ci in range(NKC)], axis=1)

        in_maps.append({
            "packA": np.ascontiguousarray(packA),
            "packB": np.ascontiguousarray(packB),
            "pack2": np.ascontiguousarray(pack2),
            "wb": np.ascontiguousarray(wb),
            "maskrow": mask,
            "z32": z32,
        })
    return KC, in_maps


def assemble_out(results):
    out = np.empty((B, Q, V), np.float32)
    for c in range(NCORES):
        b, half = divmod(c, 2)
        out[b, half * QS:(half + 1) * QS, :] = results[c]["out"]
    return out


def kernel(**inputs):
    KC, in_maps = prepare_in_maps(**inputs)
    nc = _get_nc(KC)
    try:
        res = run_bass_kernel_spmd(nc, in_maps, list(range(NCORES))).results
    except Exception:
        # transient NRT/device hiccups happen; one retry
        import time
        time.sleep(2.0)
        res = run_bass_kernel_spmd(nc, in_maps, list(range(NCORES))).results
    return assemble_out(res)


if __name__ == "__main__":
    rng = np.random.default_rng(0)
    inp = {
        "queries": rng.standard_normal((B, Q, D), np.float32),
        "keys": rng.standard_normal((B, K, D), np.float32),
        "values": rng.standard_normal((B, K, V), np.float32),
        "valid_lens": rng.integers(1, K + 1, (B,)).astype(np.int32),
        "wq_w": (rng.standard_normal((H, D), np.float32) / np.sqrt(D)).astype(np.float32),
        "wq_b": np.zeros((H,), np.float32),
        "wk_w": (rng.standard_normal((H, D), np.float32) / np.sqrt(D)).astype(np.float32),
        "wk_b": np.zeros((H,), np.float32),
        "wv_w": (rng.standard_normal((1, H), np.float32) / np.sqrt(H)).astype(np.float32),
        "wv_b": np.zeros((1,), np.float32),
    }
    out = kernel(**inp)
    print("kernel output", out.shape, out.dtype, float(np.abs(out).mean()))
